# revision 1
# baseline (speedup 1.0000x reference)
"""Causal self-attention on 8 axon-tunneled TRN2 NeuronCores.

Sharding: core c -> (batch b = c//2, head-group g = c%2); host sums the two
head-group partial yT outputs per batch. All matmuls float32r (full PE rate,
~1.5e-4 err). Transpose-free S^T attention layout; softmax denominator via a
ones-column in v; 1/s broadcast via K=1 PE matmul. Attention processes head
PAIRS with interleaved row-groups (0-1 / 2-3) so LDWEIGHTS of one head's
scores overlaps the other head's matmuls."""
import numpy as np

B, T, D = 4, 2048, 1024
NH_LOCAL = 8
HD = 64
CL = 512
P = 128
CC = D // P
TC = T // P
TT = T // 512
NPAIR = 4

_CACHE = {}


def _emit_qkv(nc, tc, mybir, r, aps, qT_sb, kT_sb, v_sb):
    f32 = mybir.dt.float32
    f32r = mybir.dt.float32r
    xT_r, wqT_r, wkT_r, wvT_r = aps
    with tc.tile_pool(name=f"p1x{r}", bufs=1) as p1x, \
         tc.tile_pool(name=f"p1q{r}", bufs=3, space="PSUM") as p1q, \
         tc.tile_pool(name=f"p1ps{r}", bufs=2, space="PSUM") as p1ps:
        xT_sb = p1x.tile([P, CC, T], f32r, tag="xT")

        # qT / kT first: PE can start as soon as the first weight slice lands
        with tc.tile_pool(name=f"p1w{r}", bufs=2) as p1w:
            w_slices = []
            for p_i in range(2):  # prefetch first two pairs' weights
                for w_r, wtag in ((wqT_r, "wq"), (wkT_r, "wk")):
                    w_sl = p1w.tile([P, CC, P], f32r, tag=f"{wtag}{p_i}")
                    nc.sync.dma_start(
                        w_sl[:], w_r[:, :, p_i * P:(p_i + 1) * P])
                    w_slices.append(w_sl)
            # xT on two queues for 2x DMA bandwidth; chunk 0 first on the
            # gpsimd queue so it lands in parallel with the weight slices
            for cc in range(CC):
                eng = nc.gpsimd if cc % 2 == 0 else nc.sync
                eng.dma_start(xT_sb[:, cc, :], xT_r[:, cc, :])
            for p_i in range(NPAIR):
                for wi, (w_r, dst, wtag) in enumerate(
                        ((wqT_r, qT_sb, "wq"), (wkT_r, kT_sb, "wk"))):
                    if p_i < 2:
                        w_sl = w_slices[p_i * 2 + wi]
                    else:
                        w_sl = p1w.tile([P, CC, P], f32r,
                                        tag=f"{wtag}{p_i % 2}")
                        nc.sync.dma_start(
                            w_sl[:], w_r[:, :, p_i * P:(p_i + 1) * P])
                    for half in range(2):
                        pq = p1q.tile([P, 1024], f32, tag="pq")
                        for s5 in range(2):
                            for cc in range(CC):
                                nc.tensor.matmul(
                                    pq[:, s5 * 512:(s5 + 1) * 512],
                                    w_sl[:, cc, :],
                                    xT_sb[:, cc,
                                          half * 1024 + s5 * 512:
                                          half * 1024 + (s5 + 1) * 512],
                                    start=(cc == 0), stop=(cc == CC - 1))
                        nc.vector.tensor_copy(
                            dst[:, p_i, half * 1024:(half + 1) * 1024],
                            pq[:])

        # v = x @ wv^T in [t, c_local] layout (wv DMA overlaps q/k compute)
        with tc.tile_pool(name=f"p1wv{r}", bufs=1) as p1wv:
            wv_sb = p1wv.tile([P, CC, CL], f32r, tag="wv")
            for cc in range(CC):
                nc.gpsimd.dma_start(wv_sb[:, cc, :], wvT_r[:, cc, :])
            for t_c in range(TC):
                pv = p1ps.tile([P, CL], f32, tag="pv")
                for cc in range(CC):
                    nc.tensor.matmul(
                        pv[:],
                        xT_sb[:, cc, t_c * P:(t_c + 1) * P],
                        wv_sb[:, cc, :],
                        start=(cc == 0), stop=(cc == CC - 1))
                nc.vector.tensor_copy(
                    v_sb[:, t_c, :, 0:HD],
                    pv[:].rearrange("p (h d) -> p h d", h=NH_LOCAL))


def _emit_attention(nc, tc, mybir, r, qT_sb, kT_sb, v_sb, tri_sb, ones_sb, aT_sb):
    """Per (head, i-half) units. aT psum is [65, 1024] (2 banks) double
    buffered, so consecutive units overlap on PE/ACT while the previous
    unit's normalize drains."""
    f32 = mybir.dt.float32
    f32r = mybir.dt.float32r
    EXP = mybir.ActivationFunctionType.Exp
    MULT = mybir.AluOpType.mult
    with tc.tile_pool(name=f"p2{r}", bufs=2) as p2, \
         tc.tile_pool(name=f"p2pt{r}", bufs=4) as p2pt, \
         tc.tile_pool(name=f"p2d{r}", bufs=4, space="DRAM") as p2d, \
         tc.tile_pool(name=f"p2aps{r}", bufs=1, space="PSUM") as p2aps, \
         tc.tile_pool(name=f"p2sps{r}", bufs=2, space="PSUM") as p2sps:
        def _normalize(aT_ps, p_i, par, h0):
            # 1/s row -> broadcast to 64 partitions via K=1 PE matmuls
            # (ones column x reciprocal row), then one multiply into aT_sb.
            rr = p2.tile([P, 1024], f32r, tag="rr")
            with nc.allow_low_precision(reason="f32r recip feeds f32r matmul"):
                nc.vector.reciprocal(rr[64:65, :], aT_ps[64:65, :])
            bc = p2sps.tile([P, 1024], f32, tag="st")
            for s5 in range(0, 1024, 512):
                nc.tensor.matmul(bc[0:HD, s5:s5 + 512], ones_sb[64:65, :],
                                 rr[64:65, s5:s5 + 512], start=True, stop=True)
            rb = p2.tile([HD, 1024], f32, tag="rb")
            nc.vector.tensor_copy(rb[:], bc[0:HD, :])
            if par == 0:
                nc.vector.tensor_tensor(
                    aT_sb[0:HD, p_i, h0:h0 + 1024],
                    aT_ps[0:HD, :], rb[:], MULT)
            else:
                t64 = p2.tile([HD, 1024], f32r, tag="t64")
                nc.vector.tensor_tensor(
                    t64[:], aT_ps[0:HD, :], rb[:], MULT)
                nc.sync.dma_start(
                    aT_sb[HD:P, p_i, h0:h0 + 1024], t64[:])

        for p_i in range(NPAIR):
            for half in range(2):
                h0, h1 = half * 1024, (half + 1) * 1024
                aTs = [p2aps.tile([HD + 1, 1024], f32, tag=f"aT{e}",
                                  name=f"aT{e}_{p_i}_{half}")
                       for e in range(2)]
                jc_end = 8 if half == 0 else 16
                for jc in range(jc_end):
                    w0 = max(h0, 512 * (jc // 4))
                    off = max(0, P * jc - w0)
                    wlen = h1 - w0
                    pts, sts = [], []
                    # scores for both heads back-to-back: alternating PE row
                    # groups (0-1 vs 2-3) let LDWEIGHTS pull ahead
                    for par in range(2):
                        prow = 64 * par
                        st = p2sps.tile([P, 1024], f32, tag="st")
                        for s5 in range(0, wlen, 512):
                            nc.tensor.matmul(
                                st[:, s5:s5 + 512],
                                kT_sb[prow:prow + HD, p_i,
                                      jc * P:(jc + 1) * P],
                                qT_sb[prow:prow + HD, p_i,
                                      w0 + s5:w0 + s5 + 512],
                                start=True, stop=True)
                        sts.append(st)
                    for par in range(2):
                        pt = p2pt.tile([P, 1024], f32r, tag="pt")
                        if off:
                            nc.vector.memset(pt[:, :off].bitcast(f32), 0.0)
                        nc.scalar.activation(
                            pt[:, off:wlen], sts[par][:, off:wlen],
                            EXP, scale=0.125)
                        if off or P * jc == w0:
                            nc.vector.tensor_tensor(
                                pt[:, off:off + P], pt[:, off:off + P],
                                tri_sb[:], MULT)
                        pts.append(pt)
                    for par in range(2):
                        h = 2 * p_i + par
                        for s5 in range(0, wlen, 512):
                            i0 = w0 + s5
                            it = i0 // 512
                            nc.tensor.matmul(
                                aTs[par][:, i0 - h0:i0 - h0 + 512],
                                v_sb[:, jc, h, :],
                                pts[par][:, s5:s5 + 512],
                                start=(jc == 0), stop=(jc == 4 * it + 3))
                for par in range(2):
                    _normalize(aTs[par], p_i, par, h0)


def _emit_out_proj(nc, tc, mybir, r, yT_r, aT_sb, wo_sb):
    f32 = mybir.dt.float32
    with tc.tile_pool(name=f"p3{r}", bufs=4) as p3, \
         tc.tile_pool(name=f"p3ps{r}", bufs=4, space="PSUM") as p3ps:
        for fc in range(CC):
            for tt in range(TT):
                py = p3ps.tile([P, 512], f32, tag="py")
                for cc in range(NPAIR):
                    nc.tensor.matmul(
                        py[:],
                        wo_sb[:, cc, fc * P:(fc + 1) * P],
                        aT_sb[:, cc, tt * 512:(tt + 1) * 512],
                        start=(cc == 0), stop=(cc == NPAIR - 1))
                yst = p3.tile([P, 512], f32, tag="yst")
                nc.vector.tensor_copy(yst[:], py[:])
                eng = nc.sync if (fc * TT + tt) % 2 == 0 else nc.gpsimd
                eng.dma_start(
                    yT_r[:, fc, tt * 512:(tt + 1) * 512], yst[:])


def _build(repeats=1):
    import concourse.bacc as bacc
    import concourse.mybir as mybir
    import concourse.tile as tile
    from contextlib import ExitStack

    f32 = mybir.dt.float32
    f32r = mybir.dt.float32r

    nc = bacc.Bacc("TRN2", target_bir_lowering=False, debug=False)

    xT = nc.dram_tensor("xT", (D, T), f32r, kind="ExternalInput")
    wqT = nc.dram_tensor("wqT", (D, CL), f32r, kind="ExternalInput")
    wkT = nc.dram_tensor("wkT", (D, CL), f32r, kind="ExternalInput")
    wvT = nc.dram_tensor("wvT", (D, CL), f32r, kind="ExternalInput")
    woT = nc.dram_tensor("woT", (CL, D), f32r, kind="ExternalInput")
    tri = nc.dram_tensor("tri", (P, P), f32, kind="ExternalInput")
    yT = nc.dram_tensor("yT", (D, T), f32, kind="ExternalOutput")

    xT_r = xT.ap().rearrange("(o p) t -> p o t", p=P)
    wqT_r = wqT.ap().rearrange("(o p) f -> p o f", p=P)
    wkT_r = wkT.ap().rearrange("(o p) f -> p o f", p=P)
    wvT_r = wvT.ap().rearrange("(o p) f -> p o f", p=P)
    woT_r = woT.ap().rearrange("(o p) f -> p o f", p=P)
    yT_r = yT.ap().rearrange("(o p) t -> p o t", p=P)

    with tile.TileContext(nc) as tc, ExitStack() as outer:
        persist = outer.enter_context(tc.tile_pool(name="persist", bufs=1))
        qT_sb = persist.tile([P, NPAIR, T], f32r, tag="qT")
        kT_sb = persist.tile([P, NPAIR, T], f32r, tag="kT")
        v_sb = persist.tile([P, TC, NH_LOCAL, HD + 1], f32r, tag="v")
        tri_sb = persist.tile([P, P], f32, tag="tri")
        nc.sync.dma_start(tri_sb[:], tri.ap())
        ones_sb = persist.tile([P, HD], f32r, tag="ones")
        nc.vector.memset(ones_sb[:].bitcast(f32), 1.0)

        for r in range(repeats):
            nc.vector.memset(v_sb[:, :, :, HD:HD + 1].bitcast(f32), 1.0)
            _emit_qkv(nc, tc, mybir, r, (xT_r, wqT_r, wkT_r, wvT_r),
                      qT_sb, kT_sb, v_sb)
            with tc.tile_pool(name=f"aT{r}", bufs=1) as aTp, \
                 tc.tile_pool(name=f"wo{r}", bufs=1) as wop:
                aT_sb = aTp.tile([P, NPAIR, T], f32r, tag="aT")
                wo_sb = wop.tile([P, NPAIR, D], f32r, tag="wo")
                nc.sync.dma_start(wo_sb[:], woT_r)
                _emit_attention(nc, tc, mybir, r, qT_sb, kT_sb, v_sb,
                                tri_sb, ones_sb, aT_sb)
                _emit_out_proj(nc, tc, mybir, r, yT_r, aT_sb, wo_sb)

    nc.compile()
    return nc


def kernel(x, w_qkv, w_out):
    from concourse import bass_utils

    if "nc" not in _CACHE:
        _CACHE["nc"] = _build()
    nc = _CACHE["nc"]

    x = np.asarray(x, dtype=np.float32)
    w_qkv = np.asarray(w_qkv, dtype=np.float32)
    w_out = np.asarray(w_out, dtype=np.float32)
    tri = np.triu(np.ones((P, P), dtype=np.float32))

    in_maps = []
    for c in range(8):
        b, g = c // 2, c % 2
        sl = slice(CL * g, CL * g + CL)
        in_maps.append({
            "xT": np.ascontiguousarray(x[b].T),
            "wqT": np.ascontiguousarray(w_qkv[0 * D:1 * D][sl].T),
            "wkT": np.ascontiguousarray(w_qkv[1 * D:2 * D][sl].T),
            "wvT": np.ascontiguousarray(w_qkv[2 * D:3 * D][sl].T),
            "woT": np.ascontiguousarray(w_out[:, sl].T),
            "tri": tri,
        })

    res = bass_utils.run_bass_kernel_spmd(nc, in_maps, core_ids=list(range(8)))
    outs = res.results

    y = np.empty((B, T, D), dtype=np.float32)
    for b in range(B):
        y[b] = (outs[2 * b]["yT"] + outs[2 * b + 1]["yT"]).T
    return y

